# revision 34
# baseline (speedup 1.0000x reference)
"""Trainium2 8-core kernel for nn_Attention_13134009991266.

Multi-head attention (B=16, L=1024, D=512, H=8, Dh=64) with a gathered
relative-position bias table, softmax, and output projection.

Sharding: data-parallel over batch (2 batches per core). The bias matrix
bias[h,i,j] = table[h, coords[i,j]] is shared by all cores: each core
gathers 1/8 of exp(bias) (its 128-row j-slab, via the native GPSIMD
indirect_copy — ~27ns/index hardware floor, so the j-sharded gather is
the critical path), and an AllGather distributes the full exp-bias to
every core. Softmax uses the factored form
  softmax(qk/s + bias) = exp(qk/s) * exp(bias) / sum(...)
so the bias-add becomes a cheap bf16 multiply on DVE and exp(bias) is
fused into the gather's f32->bf16 evacuation on ScalarE. The row-sum
denominator comes from 64 replicated ones-columns prepended to v (ones
first, so the reciprocal_approx_fast input starts at partition 0 — it
returns garbage for partition-offset inputs). Attention is software-
pipelined: AV matmuls trail the score/exp/mult chain by one j-tile, and
exp-bias tiles prefetch on the otherwise-idle GPSIMD dispatch queue.
"""
import sys
import numpy as np

sys.path.insert(0, "/opt/trn_rl_repo")

B, L, D = 16, 1024, 512
H, DH = 8, 64
NUM_REL = 3969
N_CORES = 8
BPC = B // N_CORES          # batches per core
T = BPC * L                 # tokens per core (2048)
JT = L // 128               # j tiles (8)
IC = L // 512               # i chunks per batch (2)
SLAB = L // N_CORES         # j rows gathered per core (128)

_compiled = None


def _build():
    from concourse import bass, bacc, tile, mybir

    F32 = mybir.dt.float32
    BF16 = mybir.dt.bfloat16
    U16 = mybir.dt.uint16
    AF = mybir.ActivationFunctionType
    ALU = mybir.AluOpType

    nc = bacc.Bacc("TRN2", target_bir_lowering=False, debug=False,
                   num_devices=N_CORES)

    xT_e = nc.declare_dram_parameter("xT", [D, T], BF16, isOutput=False)
    wq_e = nc.declare_dram_parameter("wq", [D, D], BF16, isOutput=False)
    wk_e = nc.declare_dram_parameter("wk", [D, D], BF16, isOutput=False)
    wv_e = nc.declare_dram_parameter("wv", [D, D], BF16, isOutput=False)
    wo_e = nc.declare_dram_parameter("wo", [D, D], BF16, isOutput=False)
    bo_e = nc.declare_dram_parameter("bo", [128, 4], F32, isOutput=False)
    tbl_e = nc.declare_dram_parameter("tbl", [128, NUM_REL], F32, isOutput=False)
    idx_e = nc.declare_dram_parameter("idx", [128, 1024], U16, isOutput=False)
    out_e = nc.declare_dram_parameter("out", [D, T], F32, isOutput=True)

    with tile.TileContext(nc) as tc:
        with tc.tile_pool(name="w", bufs=1) as wp, \
             tc.tile_pool(name="acts", bufs=1) as ap_, \
             tc.tile_pool(name="gatf", bufs=2) as gfp, \
             tc.tile_pool(name="gatb", bufs=2) as gbp, \
             tc.tile_pool(name="att", bufs=4) as atp, \
             tc.tile_pool(name="psA", bufs=2, space="PSUM") as psA, \
             tc.tile_pool(name="psO", bufs=4, space="PSUM") as psO, \
             tc.tile_pool(name="dram", bufs=1, space="DRAM") as dp:

            # ---- load table/indices first: the gather critical path starts here ----
            tbl = wp.tile([128, NUM_REL], F32, tag="tbl")
            for sl in range(8):
                nc.sync.dma_start(out=tbl[16 * sl:16 * (sl + 1), :],
                                  in_=tbl_e[16 * sl:16 * (sl + 1), :])
            idx = wp.tile([128, 1024], U16, tag="idx")
            nc.sync.dma_start(out=idx[:, :], in_=idx_e[:, :])

            # ---- load weights/acts; dispatched from the scalar queue so the
            # first gather's sync-queue wait covers only tbl+idx ----
            wq = [wp.tile([128, D], BF16, tag=f"wq{m}", name=f"wq{m}") for m in range(4)]
            wk = [wp.tile([128, D], BF16, tag=f"wk{m}", name=f"wk{m}") for m in range(4)]
            wv = [wp.tile([128, D], BF16, tag=f"wv{m}", name=f"wv{m}") for m in range(4)]
            wo = [wp.tile([128, D], BF16, tag=f"wo{m}", name=f"wo{m}") for m in range(4)]
            for wt, we in ((wq, wq_e), (wk, wk_e), (wv, wv_e), (wo, wo_e)):
                for m in range(4):
                    nc.scalar.dma_start(out=wt[m][:, :],
                                        in_=we[128 * m:128 * (m + 1), :])
            bo = wp.tile([128, 4], F32, tag="bo")
            nc.scalar.dma_start(out=bo[:, :], in_=bo_e[:, :])
            xT = [ap_.tile([128, T], BF16, tag=f"xT{m}", name=f"xT{m}") for m in range(4)]
            for m in range(4):
                nc.scalar.dma_start(out=xT[m][:, :],
                                    in_=xT_e[128 * m:128 * (m + 1), :])


            # ---- sharded gather of exp(bias) for this core's j-slab ----
            # slab layout in DRAM keeps each gather partition's 4KB run
            # contiguous: agin[h, e, c, jl, i] with j_local = 16e + 2c + jl
            agin = dp.tile([H, 8, 8, 2, L], BF16)
            agout = dp.tile([N_CORES, H, 8, 8, 2, L], BF16, addr_space="Shared")
            NE = 8          # gather eighths
            SLOTE = 2048    # slots per Q7 core per eighth
            for e in range(NE):
                gb = gbp.tile([128, SLOTE], BF16, tag="gb")
                for hh in range(2):
                    gf = gfp.tile([128, 1024], F32, tag="gf")
                    # native hardware-indirect gather: out[p, s] = tbl[p, idx(s)]
                    # (same 16-partition index wrapping as ap_gather, but avoids
                    # ap_gather's ~102-cycle-per-4-idx Q7 read commands; codegen
                    # caps num_idxs at 1024 per call)
                    nc.gpsimd.indirect_copy(
                        gf[:, :], tbl[:, :],
                        idx[:, 128 * e + 64 * hh:128 * e + 64 * (hh + 1)],
                        i_know_ap_gather_is_preferred=True,
                    )
                    # fused exp + f32->bf16 cast on ScalarE
                    nc.scalar.activation(gb[:, 1024 * hh:1024 * (hh + 1)],
                                         gf[:, :], AF.Exp)
                for c in range(8):
                    src = gb[16 * c:16 * c + 8, :].rearrange(
                        "h (jl i) -> h jl i", jl=2)
                    nc.sync.dma_start(out=agin[:, e, c], in_=src)

            nc.gpsimd.collective_compute(
                "AllGather", ALU.bypass,
                replica_groups=[list(range(N_CORES))],
                ins=[agin.opt()], outs=[agout.opt()],
            )

            # ---- projections ----
            # qT[d,t] (scaled later in exp), kT[d,t]: lhsT=w[c,d] rhs=xT[c,t]
            qT = [ap_.tile([128, T], BF16, tag=f"qT{m}", name=f"qT{m}") for m in range(4)]
            kT = [ap_.tile([128, T], BF16, tag=f"kT{m}", name=f"kT{m}") for m in range(4)]
            for m in range(4):
                for ch in range(4):
                    ps = psA.tile([128, 512], F32, tag="att")
                    for kt in range(4):
                        nc.tensor.matmul(
                            ps[:, :],
                            lhsT=wq[kt][:, 128 * m:128 * (m + 1)],
                            rhs=xT[kt][:, 512 * ch:512 * (ch + 1)],
                            start=(kt == 0), stop=(kt == 3))
                    nc.vector.tensor_copy(
                        qT[m][:, 512 * ch:512 * (ch + 1)], ps[:, :])
                    ps2 = psA.tile([128, 512], F32, tag="att")
                    for kt in range(4):
                        nc.tensor.matmul(
                            ps2[:, :],
                            lhsT=wk[kt][:, 128 * m:128 * (m + 1)],
                            rhs=xT[kt][:, 512 * ch:512 * (ch + 1)],
                            start=(kt == 0), stop=(kt == 3))
                    nc.scalar.activation(
                        kT[m][:, 512 * ch:512 * (ch + 1)],
                        ps2[:, :], AF.Copy)

            # v in token-major with per-head [64 v | 64 ones] blocks:
            # vaug[t, 128h:128h+64) = v_h, [128h+64,128h+128) = 1.0
            vaug = [ap_.tile([128, 1024], BF16, tag=f"vaug{tj}", name=f"vaug{tj}")
                    for tj in range(T // 128)]
            for tj in range(T // 128):
                vt = vaug[tj][:, :]
                # ones (softmax denominator) in the FIRST half of each head
                # block so the divide's reciprocal input starts at partition 0
                # (reciprocal_approx_fast breaks on partition-offset inputs)
                nc.vector.memset(
                    vt.rearrange("p (h two d) -> p h two d", h=8, two=2)[:, :, 0, :],
                    1.0)
                ps = psA.tile([128, 512], F32, tag="att")
                for kt in range(4):
                    nc.tensor.matmul(
                        ps[:, :],
                        lhsT=xT[kt][:, 128 * tj:128 * (tj + 1)],
                        rhs=wv[kt][:, :],
                        start=(kt == 0), stop=(kt == 3))
                nc.vector.tensor_copy(
                    vt.rearrange("p (h two d) -> p h two d", h=8, two=2)[:, :, 1, :],
                    ps.rearrange("p (h d) -> p h d", h=8))

            # ---- attention: scores, exp, mult by exp(bias), attnv, divide ----
            oT = [ap_.tile([128, T], BF16, tag=f"oT{m}", name=f"oT{m}") for m in range(4)]

            def outproj(chs):
                # out[e,t] = wo.T @ oT + bo for token chunks chs
                for m in range(4):
                    for ch in chs:
                        ps = psA.tile([128, 512], F32, tag="att")
                        for kt in range(4):
                            nc.tensor.matmul(
                                ps[:, :],
                                lhsT=wo[kt][:, 128 * m:128 * (m + 1)],
                                rhs=oT[kt][:, 512 * ch:512 * (ch + 1)],
                                start=(kt == 0), stop=(kt == 3))
                        ot = atp.tile([128, 512], F32, tag="outev")
                        nc.scalar.activation(ot[:, :], ps[:, :], AF.Identity,
                                             bias=bo[:, m:m + 1])
                        nc.sync.dma_start(
                            out=out_e[128 * m:128 * (m + 1),
                                      512 * ch:512 * (ch + 1)],
                            in_=ot[:, :])

            for hp in range(4):           # head pair (2hp, 2hp+1)
                for ic in range(2):
                    pso = [[psO.tile([128, 512], F32, tag="pso",
                                     name=f"pso{hp}_{ic}_{s2}_{b2}")
                            for b2 in range(BPC)] for s2 in range(2)]
                    # et prefetch on the (idle) GPSIMD dispatch queue, one
                    # j-tile ahead; AV matmuls deferred one j-tile so TensorE
                    # can run scores ahead of the exp->mult chain.
                    ets = {}

                    def load_et(t, hp=hp, ic=ic, ets=ets):
                        for s in range(2):
                            et = atp.tile([128, 512], BF16, tag="et",
                                          name=f"et{hp}_{ic}_{t}_{s}")
                            nc.gpsimd.dma_start(
                                out=et[:, :],
                                in_=agout[t, 2 * hp + s].rearrange(
                                    "e c jl i -> (e c jl) i")[
                                    :, 512 * ic:512 * (ic + 1)])
                            ets[(t, s)] = et

                    def av_mms(tp, s, exq, last, hp=hp):
                        for b in range(BPC):
                            nc.tensor.matmul(
                                pso[s][b][:, :],
                                lhsT=vaug[8 * b + tp][:,
                                          128 * (2 * hp + s):128 * (2 * hp + s + 1)],
                                rhs=exq[:, 512 * b:512 * (b + 1)],
                                start=(tp == 0), stop=last)

                    load_et(0)
                    pend = []
                    for t in range(JT):
                        if t + 1 < JT:
                            load_et(t + 1)
                        cur = []
                        for s in range(2):    # head-in-pair
                            et = ets.pop((t, s))
                            ps = psA.tile([128, 1024], F32, tag="att")
                            for b in range(BPC):
                                nc.tensor.matmul(
                                    ps[:, 512 * b:512 * (b + 1)],
                                    lhsT=kT[hp][64 * s:64 * (s + 1),
                                            1024 * b + 128 * t:1024 * b + 128 * (t + 1)],
                                    rhs=qT[hp][64 * s:64 * (s + 1),
                                           1024 * b + 512 * ic:1024 * b + 512 * (ic + 1)],
                                    start=True, stop=True,
                                    tile_position=(64 * s, 0))
                            ex = atp.tile([128, 1024], BF16, tag="ex",
                                          name=f"ex{hp}_{ic}_{t}_{s}")
                            nc.scalar.activation(ex[:, :], ps[:, :], AF.Exp,
                                                 scale=0.125)
                            etb = et[:, :].rearrange(
                                "p (one i) -> p one i", one=1).broadcast_to(
                                [128, 2, 512])
                            nc.vector.tensor_tensor(
                                ex[:, :].rearrange("p (b i) -> p b i", b=2),
                                ex[:, :].rearrange("p (b i) -> p b i", b=2),
                                etb, ALU.mult)
                            cur.append((t, s, ex))
                        for (tp, s, exq) in pend:
                            av_mms(tp, s, exq, False)
                        pend = cur
                    for (tp, s, exq) in pend:
                        av_mms(tp, s, exq, True)
                    for s in range(2):
                        for b in range(BPC):
                            rc = atp.tile([64, 512], F32, tag="rc")
                            nc.vector.reciprocal_approx_fast(
                                rc[:, :], pso[s][b][0:64, :])
                            nc.vector.tensor_tensor(
                                oT[hp][64 * s:64 * (s + 1),
                                   1024 * b + 512 * ic:1024 * b + 512 * (ic + 1)],
                                pso[s][b][64:128, :], rc[:, :], ALU.mult)
            outproj([0, 1, 2, 3])

    nc.compile()
    return nc


def _get_compiled():
    global _compiled
    if _compiled is None:
        _compiled = _build()
    return _compiled


def kernel(x, rel_pos_bias, rel_pos_coords, W_q, W_k, W_v, W_o, b_o):
    import ml_dtypes
    from concourse import bass_utils

    bf16 = ml_dtypes.bfloat16
    x = np.asarray(x, np.float32)
    table = np.asarray(rel_pos_bias, np.float32).reshape(H, NUM_REL)
    coords = np.asarray(rel_pos_coords).astype(np.int64)
    W_q = np.asarray(W_q, np.float32); W_k = np.asarray(W_k, np.float32)
    W_v = np.asarray(W_v, np.float32); W_o = np.asarray(W_o, np.float32)
    b_o = np.asarray(b_o, np.float32)

    nc = _get_compiled()

    wqT = np.ascontiguousarray(W_q.T).astype(bf16)
    wkT = np.ascontiguousarray(W_k.T).astype(bf16)
    wvT = np.ascontiguousarray(W_v.T).astype(bf16)
    woT = np.ascontiguousarray(W_o.T).astype(bf16)
    bo_s = np.ascontiguousarray(b_o.reshape(4, 128).T)
    tbl_s = np.ascontiguousarray(table[np.arange(128) % 8])

    in_maps = []
    for n in range(N_CORES):
        xT = np.ascontiguousarray(
            x[BPC * n:BPC * (n + 1)].reshape(T, D).T).astype(bf16)
        # gather indices for j-slab [128n, 128(n+1)):
        # Q7 core c, slot s*16+kk covers (j_inner = slot//1024, i = slot%1024),
        # j = 128n + 16c + j_inner ; idx[16c+kk, s] = coords[i, j]
        idxm = np.empty((128, 1024), np.uint16)
        for e in range(8):
            for c in range(8):
                j0 = 128 * n + 16 * e + 2 * c
                vals = coords[:, j0:j0 + 2].T.reshape(-1)  # [2 jl x 1024 i]
                idxm[16 * c:16 * c + 16, 128 * e:128 * (e + 1)] = \
                    vals.reshape(128, 16).T
        in_maps.append({
            "xT": xT, "wq": wqT, "wk": wkT, "wv": wvT, "wo": woT,
            "bo": bo_s, "tbl": tbl_s, "idx": idxm,
        })

    res = bass_utils.run_bass_kernel_spmd(
        nc, in_maps, core_ids=list(range(N_CORES)))
    out = np.empty((B, L, D), np.float32)
    for n in range(N_CORES):
        out[BPC * n:BPC * (n + 1)] = (
            res.results[n]["out"].T.reshape(BPC, L, D))
    return out



# revision 44
# speedup vs baseline: 1.0792x; 1.0792x over previous
"""Trainium2 8-core kernel for nn_Attention_13134009991266.

Multi-head attention (B=16, L=1024, D=512, H=8, Dh=64) with a gathered
relative-position bias table, softmax, and output projection.

Sharding: data-parallel over batch (2 batches per core). The bias matrix
bias[h,i,j] = table[h, coords[i,j]] is shared by all cores: each core
gathers 1/8 of exp(bias) (its 128-row j-slab, via the native GPSIMD
indirect_copy — ~27ns/index hardware floor, so the j-sharded gather is
the critical path), and an AllGather distributes the full exp-bias to
every core. Softmax uses the factored form
  softmax(qk/s + bias) = exp(qk/s) * exp(bias) / sum(...)
so the bias-add becomes a cheap bf16 multiply on DVE and exp(bias) is
fused into the gather's f32->bf16 evacuation on ScalarE. The row-sum
denominator comes from 64 replicated ones-columns prepended to v (ones
first, so the reciprocal_approx_fast input starts at partition 0 — it
returns garbage for partition-offset inputs). Attention is software-
pipelined: AV matmuls trail the score/exp/mult chain by one j-tile, and
exp-bias tiles prefetch on the otherwise-idle GPSIMD dispatch queue.
"""
import sys
import numpy as np

sys.path.insert(0, "/opt/trn_rl_repo")

B, L, D = 16, 1024, 512
H, DH = 8, 64
NUM_REL = 3969
N_CORES = 8
BPC = B // N_CORES          # batches per core
T = BPC * L                 # tokens per core (2048)
JT = L // 128               # j tiles (8)
IC = L // 512               # i chunks per batch (2)
SLAB = L // N_CORES         # j rows gathered per core (128)

_compiled = None


def _build():
    from concourse import bass, bacc, tile, mybir

    F32 = mybir.dt.float32
    BF16 = mybir.dt.bfloat16
    U16 = mybir.dt.uint16
    AF = mybir.ActivationFunctionType
    ALU = mybir.AluOpType

    nc = bacc.Bacc("TRN2", target_bir_lowering=False, debug=False,
                   num_devices=N_CORES)

    xT_e = nc.declare_dram_parameter("xT", [D, T], BF16, isOutput=False)
    wq_e = nc.declare_dram_parameter("wq", [D, D], BF16, isOutput=False)
    wk_e = nc.declare_dram_parameter("wk", [D, D], BF16, isOutput=False)
    wv_e = nc.declare_dram_parameter("wv", [D, D], BF16, isOutput=False)
    wo_e = nc.declare_dram_parameter("wo", [D, D], BF16, isOutput=False)
    bo_e = nc.declare_dram_parameter("bo", [128, 4], F32, isOutput=False)
    tbl_e = nc.declare_dram_parameter("tbl", [128, NUM_REL], F32, isOutput=False)
    idx_e = nc.declare_dram_parameter("idx", [128, 1024], U16, isOutput=False)
    out_e = nc.declare_dram_parameter("out", [D, T], F32, isOutput=True)

    with tile.TileContext(nc) as tc:
        with tc.tile_pool(name="w", bufs=1) as wp, \
             tc.tile_pool(name="acts", bufs=1) as ap_, \
             tc.tile_pool(name="gatf", bufs=2) as gfp, \
             tc.tile_pool(name="gatb", bufs=2) as gbp, \
             tc.tile_pool(name="att", bufs=4) as atp, \
             tc.tile_pool(name="psA", bufs=2, space="PSUM") as psA, \
             tc.tile_pool(name="psO", bufs=4, space="PSUM") as psO, \
             tc.tile_pool(name="dram", bufs=1, space="DRAM") as dp:

            # ---- load table/indices first: the gather critical path starts here ----
            tbl = wp.tile([128, NUM_REL], F32, tag="tbl")
            for sl in range(8):
                nc.sync.dma_start(out=tbl[16 * sl:16 * (sl + 1), :],
                                  in_=tbl_e[16 * sl:16 * (sl + 1), :])
            idx = wp.tile([128, 1024], U16, tag="idx")
            nc.sync.dma_start(out=idx[:, :], in_=idx_e[:, :])

            # ---- load weights/activations ----
            wq = [wp.tile([128, D], BF16, tag=f"wq{m}", name=f"wq{m}") for m in range(4)]
            wk = [wp.tile([128, D], BF16, tag=f"wk{m}", name=f"wk{m}") for m in range(4)]
            wv = [wp.tile([128, D], BF16, tag=f"wv{m}", name=f"wv{m}") for m in range(4)]
            wo = [wp.tile([128, D], BF16, tag=f"wo{m}", name=f"wo{m}") for m in range(4)]
            for wt, we in ((wq, wq_e), (wk, wk_e), (wv, wv_e), (wo, wo_e)):
                for m in range(4):
                    nc.sync.dma_start(out=wt[m][:, :],
                                      in_=we[128 * m:128 * (m + 1), :])
            bo = wp.tile([128, 4], F32, tag="bo")
            nc.sync.dma_start(out=bo[:, :], in_=bo_e[:, :])
            xT = [ap_.tile([128, T], BF16, tag=f"xT{m}", name=f"xT{m}") for m in range(4)]
            for m in range(4):
                nc.sync.dma_start(out=xT[m][:, :],
                                  in_=xT_e[128 * m:128 * (m + 1), :])


            # ---- sharded gather of exp(bias) for this core's j-slab ----
            # slab layout in DRAM keeps each gather partition's 4KB run
            # contiguous: agin[h, e, c, jl, i] with j_local = 16e + 2c + jl.
            # Two staging tiles split 6/2 along e so the first (12MB-output)
            # AllGather hides under the remaining gather calls and only a
            # 4MB AllGather stays serial after the last gather.
            aginA = dp.tile([H, 6, 8, 2, L], BF16)
            aginB = dp.tile([H, 2, 8, 2, L], BF16)
            agoutA = dp.tile([N_CORES, H, 6, 8, 2, L], BF16, addr_space="Shared")
            agoutB = dp.tile([N_CORES, H, 2, 8, 2, L], BF16, addr_space="Shared")
            NE = 8          # gather eighths
            SLOTE = 2048    # slots per Q7 core per eighth
            for e in range(NE):
                gb = gbp.tile([128, SLOTE], BF16, tag="gb")
                for hh in range(2):
                    gf = gfp.tile([128, 1024], F32, tag="gf")
                    # native hardware-indirect gather: out[p, s] = tbl[p, idx(s)]
                    # (same 16-partition index wrapping as ap_gather, but avoids
                    # ap_gather's ~102-cycle-per-4-idx Q7 read commands; codegen
                    # caps num_idxs at 1024 per call)
                    nc.gpsimd.indirect_copy(
                        gf[:, :], tbl[:, :],
                        idx[:, 128 * e + 64 * hh:128 * e + 64 * (hh + 1)],
                        i_know_ap_gather_is_preferred=True,
                    )
                    # fused exp + f32->bf16 cast on ScalarE
                    nc.scalar.activation(gb[:, 1024 * hh:1024 * (hh + 1)],
                                         gf[:, :], AF.Exp)
                for c in range(8):
                    src = gb[16 * c:16 * c + 8, :].rearrange(
                        "h (jl i) -> h jl i", jl=2)
                    if e < 6:
                        nc.sync.dma_start(out=aginA[:, e, c], in_=src)
                    else:
                        nc.sync.dma_start(out=aginB[:, e - 6, c], in_=src)
                if e == 5:
                    nc.gpsimd.collective_compute(
                        "AllGather", ALU.bypass,
                        replica_groups=[list(range(N_CORES))],
                        ins=[aginA.opt()], outs=[agoutA.opt()],
                    )

            nc.gpsimd.collective_compute(
                "AllGather", ALU.bypass,
                replica_groups=[list(range(N_CORES))],
                ins=[aginB.opt()], outs=[agoutB.opt()],
            )

            # ---- projections ----
            # qT[d,t] (scaled later in exp), kT[d,t]: lhsT=w[c,d] rhs=xT[c,t]
            qT = [ap_.tile([128, T], BF16, tag=f"qT{m}", name=f"qT{m}") for m in range(4)]
            kT = [ap_.tile([128, T], BF16, tag=f"kT{m}", name=f"kT{m}") for m in range(4)]
            for m in range(4):
                for ch in range(4):
                    ps = psA.tile([128, 512], F32, tag="att")
                    for kt in range(4):
                        nc.tensor.matmul(
                            ps[:, :],
                            lhsT=wq[kt][:, 128 * m:128 * (m + 1)],
                            rhs=xT[kt][:, 512 * ch:512 * (ch + 1)],
                            start=(kt == 0), stop=(kt == 3))
                    nc.vector.tensor_copy(
                        qT[m][:, 512 * ch:512 * (ch + 1)], ps[:, :])
                    ps2 = psA.tile([128, 512], F32, tag="att")
                    for kt in range(4):
                        nc.tensor.matmul(
                            ps2[:, :],
                            lhsT=wk[kt][:, 128 * m:128 * (m + 1)],
                            rhs=xT[kt][:, 512 * ch:512 * (ch + 1)],
                            start=(kt == 0), stop=(kt == 3))
                    nc.scalar.activation(
                        kT[m][:, 512 * ch:512 * (ch + 1)],
                        ps2[:, :], AF.Copy)

            # v in token-major with per-head [64 v | 64 ones] blocks:
            # vaug[t, 128h:128h+64) = v_h, [128h+64,128h+128) = 1.0
            vaug = [ap_.tile([128, 1024], BF16, tag=f"vaug{tj}", name=f"vaug{tj}")
                    for tj in range(T // 128)]
            for tj in range(T // 128):
                vt = vaug[tj][:, :]
                # ones (softmax denominator) in the FIRST half of each head
                # block so the divide's reciprocal input starts at partition 0
                # (reciprocal_approx_fast breaks on partition-offset inputs)
                nc.vector.memset(
                    vt.rearrange("p (h two d) -> p h two d", h=8, two=2)[:, :, 0, :],
                    1.0)
                ps = psA.tile([128, 512], F32, tag="att")
                for kt in range(4):
                    nc.tensor.matmul(
                        ps[:, :],
                        lhsT=xT[kt][:, 128 * tj:128 * (tj + 1)],
                        rhs=wv[kt][:, :],
                        start=(kt == 0), stop=(kt == 3))
                nc.vector.tensor_copy(
                    vt.rearrange("p (h two d) -> p h two d", h=8, two=2)[:, :, 1, :],
                    ps.rearrange("p (h d) -> p h d", h=8))

            # ---- attention: scores, exp, mult by exp(bias), attnv, divide ----
            oT = [ap_.tile([128, T], BF16, tag=f"oT{m}", name=f"oT{m}") for m in range(4)]

            def outproj(chs):
                # out[e,t] = wo.T @ oT + bo for token chunks chs
                for m in range(4):
                    for ch in chs:
                        ps = psA.tile([128, 512], F32, tag="att")
                        for kt in range(4):
                            nc.tensor.matmul(
                                ps[:, :],
                                lhsT=wo[kt][:, 128 * m:128 * (m + 1)],
                                rhs=oT[kt][:, 512 * ch:512 * (ch + 1)],
                                start=(kt == 0), stop=(kt == 3))
                        ot = atp.tile([128, 512], F32, tag="outev")
                        nc.scalar.activation(ot[:, :], ps[:, :], AF.Identity,
                                             bias=bo[:, m:m + 1])
                        nc.sync.dma_start(
                            out=out_e[128 * m:128 * (m + 1),
                                      512 * ch:512 * (ch + 1)],
                            in_=ot[:, :])

            for hp in range(4):           # head pair (2hp, 2hp+1)
                for ic in range(2):
                    pso = [[psO.tile([128, 512], F32, tag="pso",
                                     name=f"pso{hp}_{ic}_{s2}_{b2}")
                            for b2 in range(BPC)] for s2 in range(2)]
                    # et prefetch on the (idle) GPSIMD dispatch queue, one
                    # j-tile ahead; AV matmuls deferred one j-tile so TensorE
                    # can run scores ahead of the exp->mult chain.
                    ets = {}

                    def load_et(t, hp=hp, ic=ic, ets=ets):
                        for s in range(2):
                            h = 2 * hp + s
                            et = atp.tile([128, 512], BF16, tag="et",
                                          name=f"et{hp}_{ic}_{t}_{s}")
                            nc.gpsimd.dma_start(
                                out=et[0:96, :],
                                in_=agoutA[t, h].rearrange(
                                    "e c jl i -> (e c jl) i")[
                                    :, 512 * ic:512 * (ic + 1)])
                            nc.gpsimd.dma_start(
                                out=et[96:128, :],
                                in_=agoutB[t, h].rearrange(
                                    "e c jl i -> (e c jl) i")[
                                    :, 512 * ic:512 * (ic + 1)])
                            ets[(t, s)] = et

                    def av_mms(tp, s, exq, last, hp=hp):
                        for b in range(BPC):
                            nc.tensor.matmul(
                                pso[s][b][:, :],
                                lhsT=vaug[8 * b + tp][:,
                                          128 * (2 * hp + s):128 * (2 * hp + s + 1)],
                                rhs=exq[:, 512 * b:512 * (b + 1)],
                                start=(tp == 0), stop=last)

                    load_et(0)
                    pend = []
                    for t in range(JT):
                        if t + 1 < JT:
                            load_et(t + 1)
                        cur = []
                        for s in range(2):    # head-in-pair
                            et = ets.pop((t, s))
                            ps = psA.tile([128, 1024], F32, tag="att")
                            for b in range(BPC):
                                nc.tensor.matmul(
                                    ps[:, 512 * b:512 * (b + 1)],
                                    lhsT=kT[hp][64 * s:64 * (s + 1),
                                            1024 * b + 128 * t:1024 * b + 128 * (t + 1)],
                                    rhs=qT[hp][64 * s:64 * (s + 1),
                                           1024 * b + 512 * ic:1024 * b + 512 * (ic + 1)],
                                    start=True, stop=True,
                                    tile_position=(64 * s, 0))
                            ex = atp.tile([128, 1024], BF16, tag="ex",
                                          name=f"ex{hp}_{ic}_{t}_{s}")
                            nc.scalar.activation(ex[:, :], ps[:, :], AF.Exp,
                                                 scale=0.125)
                            etb = et[:, :].rearrange(
                                "p (one i) -> p one i", one=1).broadcast_to(
                                [128, 2, 512])
                            nc.vector.tensor_tensor(
                                ex[:, :].rearrange("p (b i) -> p b i", b=2),
                                ex[:, :].rearrange("p (b i) -> p b i", b=2),
                                etb, ALU.mult)
                            cur.append((t, s, ex))
                        for (tp, s, exq) in pend:
                            av_mms(tp, s, exq, False)
                        pend = cur
                    for (tp, s, exq) in pend:
                        av_mms(tp, s, exq, True)
                    for s in range(2):
                        for b in range(BPC):
                            rc = atp.tile([64, 512], F32, tag="rc")
                            nc.vector.reciprocal_approx_fast(
                                rc[:, :], pso[s][b][0:64, :])
                            nc.vector.tensor_tensor(
                                oT[hp][64 * s:64 * (s + 1),
                                   1024 * b + 512 * ic:1024 * b + 512 * (ic + 1)],
                                pso[s][b][64:128, :], rc[:, :], ALU.mult)
            outproj([0, 1, 2, 3])

    nc.compile()
    return nc


def _get_compiled():
    global _compiled
    if _compiled is None:
        _compiled = _build()
    return _compiled


def kernel(x, rel_pos_bias, rel_pos_coords, W_q, W_k, W_v, W_o, b_o):
    import ml_dtypes
    from concourse import bass_utils

    bf16 = ml_dtypes.bfloat16
    x = np.asarray(x, np.float32)
    table = np.asarray(rel_pos_bias, np.float32).reshape(H, NUM_REL)
    coords = np.asarray(rel_pos_coords).astype(np.int64)
    W_q = np.asarray(W_q, np.float32); W_k = np.asarray(W_k, np.float32)
    W_v = np.asarray(W_v, np.float32); W_o = np.asarray(W_o, np.float32)
    b_o = np.asarray(b_o, np.float32)

    nc = _get_compiled()

    wqT = np.ascontiguousarray(W_q.T).astype(bf16)
    wkT = np.ascontiguousarray(W_k.T).astype(bf16)
    wvT = np.ascontiguousarray(W_v.T).astype(bf16)
    woT = np.ascontiguousarray(W_o.T).astype(bf16)
    bo_s = np.ascontiguousarray(b_o.reshape(4, 128).T)
    tbl_s = np.ascontiguousarray(table[np.arange(128) % 8])

    in_maps = []
    for n in range(N_CORES):
        xT = np.ascontiguousarray(
            x[BPC * n:BPC * (n + 1)].reshape(T, D).T).astype(bf16)
        # gather indices for j-slab [128n, 128(n+1)):
        # Q7 core c, slot s*16+kk covers (j_inner = slot//1024, i = slot%1024),
        # j = 128n + 16c + j_inner ; idx[16c+kk, s] = coords[i, j]
        idxm = np.empty((128, 1024), np.uint16)
        for e in range(8):
            for c in range(8):
                j0 = 128 * n + 16 * e + 2 * c
                vals = coords[:, j0:j0 + 2].T.reshape(-1)  # [2 jl x 1024 i]
                idxm[16 * c:16 * c + 16, 128 * e:128 * (e + 1)] = \
                    vals.reshape(128, 16).T
        in_maps.append({
            "xT": xT, "wq": wqT, "wk": wkT, "wv": wvT, "wo": woT,
            "bo": bo_s, "tbl": tbl_s, "idx": idxm,
        })

    res = bass_utils.run_bass_kernel_spmd(
        nc, in_maps, core_ids=list(range(N_CORES)))
    out = np.empty((B, L, D), np.float32)
    for n in range(N_CORES):
        out[BPC * n:BPC * (n + 1)] = (
            res.results[n]["out"].T.reshape(BPC, L, D))
    return out

